# revision 31
# baseline (speedup 1.0000x reference)
"""HardBinaryConv Trainium2 kernel.

Computes y = conv2d(sign(x), sign(w)) for x [32,256,56,56] f32, w flat
[256*256*3*3, 1] f32, 3x3 kernel, stride 1, pad 1 (the STE forward pass of
reference.py).

Strategy: data-parallel over batch across 8 cores (4 images/core), weights
replicated. Per core: binarize x on the scalar engine (Sign) to fp8e4
(+-1/0 exact) into zero-padded 58x58 SBUF images, both 128-channel chunks
packed [128, 2, 3376] (16B-aligned stride for DoubleRow); binarize the
host-relaid-out weights to fp8. Conv = 9 accumulating fp8 DoubleRow
matmuls (256-channel contraction per pass, one per 3x3 tap) per PSUM tile
of [128 out-ch, 8 rows x 56 cols]; the rhs streams a strided [2, 8, 56]
window of the padded image, so horizontal taps are plain flat offsets and
padding columns are never computed.

The tensor engine (504 groups x 448 output rows at the fp8 DoubleRow
rate, ~47.0us) is the binding resource; the schedule holds it at 100%
from ~10us on, and everything else is arranged to shorten the lead-in,
the tail, and HBM traffic:
 - y is written as f16 (conv of +-1/0 values is an exact small integer;
   f16 holds integers exactly to 2048) and widened to f32 on the host.
 - x and w are uploaded as the high 2 bytes of each f32 (a pure
   byte-gather view = bf16 truncation, no host arithmetic). sign() of a
   truncated f32 equals sign() of the original for every normal float,
   so the device result is unchanged while input HBM traffic halves.
 - w is split into two per-oc-chunk tensors, each loaded and binarized
   in two tap-slices, so the first matmul group waits on a quarter of
   the weight bytes; the first two x chunks stream before the weights.
 - image 0 arrives in 8-row chunks whose boundaries match the 8-row
   output blocks (each sign() unlocks the next block) and its first two
   blocks run oc0-only while oc1's weights are still being binarized;
   later images use coarser chunks and alternate oc per block.
 - output staging is one SBUF tile per store so a store's dependency is
   exactly the drains that feed it; all stores are issued after every
   load is queued (input never waits on output at the DMA engines), and
   image 3 runs oc-major with a descending ladder of store sizes whose
   last three triggers leave from three different sequencers - the
   non-overlappable tail is a 4-row drain plus a 4-row store.
 - a bridge of tiny self-referential matmuls keeps the PE busy from
   t~0.5 to the first real matmul so the p-state ramp is complete.

Since all matmul operands are exactly +-1/0 (sums of <=2304 of them are
exact integers in f32 PSUM and f16 output), the result is bit-exact vs
the reference (rel err 0.0).
"""

import numpy as np

import concourse.bass as bass
import concourse.bacc as bacc
import concourse.mybir as mybir
from concourse.tile import TileContext
from concourse.bass_utils import run_bass_kernel_spmd

N_CORES = 8
N_IMG = 4          # images per core
CIN = 256
COUT = 256
H = W = 56
WP = 58            # padded width
BASE = 2           # guard elements in front of the padded image
CSTRIDE = 3376     # per-c-chunk stride in the padded tile (16B aligned for fp8)
BLK = 8            # output rows per PSUM tile
NBLK = 7           # 56 / 8
NSPAN = BLK * WP   # 464 <= 512 (one PSUM bank in f32)

# x row chunks; block b needs rows <= 8b+8. Image 0 arrives in 8-row
# pieces (each sign() unlocks the next block while the pipeline fills);
# later images use coarser chunks (fewer instructions, pipeline has slack).
ROWCHUNKS0 = [(0, 9), (9, 8), (17, 8), (25, 8), (33, 8), (41, 8), (49, 7)]
ROWCHUNKS = [(0, 9), (9, 16), (25, 16), (41, 15)]

TRACE = False          # set by test.py to get a profile
LAST_RESULTS = None    # BassKernelResults of the last run (when TRACE)

W_BF16 = True          # upload weights as truncated-f32 (bf16 byte view)
X_BF16 = True          # upload x as truncated-f32 (bf16 byte view)
Y_F16 = True           # store y as f16 (exact for this op), widen on host
N_BRIDGE = 270         # warm-up matmuls bridging t~0.5us .. first real matmul

_cache = {}


def _build_nc():
    nc = bacc.Bacc("TRN2", num_devices=N_CORES)
    f32 = mybir.dt.float32
    bdt = mybir.dt.float8e4
    xdt = mybir.dt.bfloat16 if X_BF16 else f32
    wdt = mybir.dt.bfloat16 if W_BF16 else f32
    ydt = mybir.dt.float16 if Y_F16 else f32

    x_t = nc.dram_tensor("x", [N_IMG, CIN, H, W], xdt, kind="ExternalInput")
    # host-prepped weight layout: [o-chunk, c%128, c//128, tap(3*dh+dw), o]
    w_t = nc.dram_tensor("w", [2, 128, 2, 9, 128], wdt, kind="ExternalInput")
    y_t = nc.dram_tensor("y", [N_IMG, COUT, H, W], ydt, kind="ExternalOutput")
    x_ap, w_ap, y_ap = x_t.ap(), w_t.ap(), y_t.ap()

    chunks = [(0, r0, nr) for r0, nr in ROWCHUNKS0] + [
        (n, r0, nr) for n in range(1, N_IMG) for r0, nr in ROWCHUNKS
    ]

    with TileContext(nc) as tc:
        with (
            tc.tile_pool(name="persist", bufs=1) as persist,
            tc.tile_pool(name="stq", bufs=12) as stq,
            tc.tile_pool(name="outp", bufs=1) as outp,
            tc.tile_pool(name="psum", bufs=7, space="PSUM") as psump,
            tc.tile_pool(name="psbr", bufs=1, space="PSUM") as psbr,
        ):
            # --- PE p-state warm-up bridge: tiny matmuls on a zeroed tile ---
            dz = persist.tile([128, 2, 192], bdt, name="dz")
            nc.gpsimd.memset(dz, 0.0)
            psd = psbr.tile([128, 64], f32, name="psd")
            for _ in range(N_BRIDGE):
                nc.tensor.matmul(
                    psd,
                    dz[:, :, 0:128],
                    dz[:, :, 128:192],
                    start=True,
                    stop=True,
                    perf_mode=mybir.MatmulPerfMode.DoubleRow,
                )

            # --- padded binarized images: [128, cc=2, 3376] ---
            xp = []
            for n in range(N_IMG):
                p = persist.tile([128, 2, CSTRIDE], bdt, name=f"xp_{n}")
                # zero guard/border cells: front guard + top row + row1-col0;
                # row56-col57 + bottom row + back guard; and the interleaved
                # (col57, next-row col0) pairs of interior rows
                nc.gpsimd.memset(p[:, :, 0 : BASE + WP + 1], 0.0)
                nc.gpsimd.memset(p[:, :, BASE + 57 * WP - 1 : CSTRIDE], 0.0)
                pairs = p[:, :, BASE + WP + 57 : BASE + 56 * WP + 57]
                pairs = pairs.rearrange("p k (r c) -> p k r c", c=WP)[:, :, :, 0:2]
                nc.gpsimd.memset(pairs, 0.0)
                xp.append(p)

            def load_chunk(n, r0, nr):
                src = x_ap[n].rearrange("(k p) h w -> p k h w", p=128)
                xf = stq.tile([128, 2, 16, W], xdt, name="xf", tag="xf")
                nc.sync.dma_start(xf[:, :, 0:nr], src[:, :, r0 : r0 + nr])
                return xf

            def sign_chunk(n, r0, nr, xf):
                interior = xp[n][:, :, BASE + WP + 1 : BASE + WP + 1 + H * WP]
                interior = interior.rearrange("p k (r c) -> p k r c", c=WP)[
                    :, :, :, 0:W
                ]
                nc.scalar.sign(interior[:, :, r0 : r0 + nr], xf[:, :, 0:nr])

            # lead-in critical chain: the first x chunk loads first (its sign
            # runs while the weights stream in); each per-oc weight tensor
            # arrives and is signed in two tap-halves so the first matmul of
            # a group starts as soon as its early taps are binarized
            wf = [
                persist.tile([128, 2, 9, 128], wdt, name=f"wf{oc}")
                for oc in range(2)
            ]
            wb = [
                persist.tile([128, 2, 9, 128], bdt, name=f"wb{oc}")
                for oc in range(2)
            ]
            def load_w(oc, taps):
                nc.sync.dma_start(wf[oc][:, :, taps], w_ap[oc][:, :, taps])
                nc.scalar.sign(wb[oc][:, :, taps], wf[oc][:, :, taps])

            xf0 = load_chunk(*chunks[0])
            sign_chunk(*chunks[0], xf0)
            # the second x chunk streams (and signs) before the weights so
            # neither of the first two oc0 blocks ever waits on an x sign
            sign_chunk(*chunks[1], load_chunk(*chunks[1]))
            load_w(0, slice(0, 7))
            load_w(0, slice(7, 9))
            load_w(1, slice(0, 7))
            load_w(1, slice(7, 9))
            for ch in chunks[2:]:
                sign_chunk(*ch, load_chunk(*ch))

            # output staging is split into per-store tiles (one DMA each) so
            # a store's dependency is exactly the drains that feed it, not
            # the whole image plane; the final tile of img3-oc1 is 4 rows so
            # the only non-overlappable tail is a 4-row drain + 4-row store
            def make_parts(n, oc, bounds):
                return [
                    (
                        r0,
                        nr,
                        outp.tile(
                            [128, nr, W], ydt, name=f"ob{n}_{oc}_{r0}"
                        ),
                    )
                    for r0, nr in bounds
                ]

            # --- conv: 9 accumulating tap matmuls per (img, row-range, oc) ---
            def conv_group(n, r0, nr, oc, parts, drain=None):
                ps = psump.tile([128, BLK, W], f32, name="ps", tag="ps")
                for dh in range(3):
                    for dw in range(3):
                        t = 3 * dh + dw
                        s = BASE + (r0 + dh) * WP + dw - 1
                        rhs = xp[n][
                            :, :, s : s + nr * WP
                        ].rearrange("p k (r c) -> p k r c", c=WP)[..., 1:57]
                        nc.tensor.matmul(
                            ps[:, 0:nr],
                            wb[oc][:, :, t],
                            rhs,
                            start=(t == 0),
                            stop=(t == 8),
                            perf_mode=mybir.MatmulPerfMode.DoubleRow,
                        )
                for p0, pn, tile in parts:
                    if p0 <= r0 and r0 + nr <= p0 + pn:
                        dst = tile[:, r0 - p0 : r0 - p0 + nr, :]
                        if drain is None:
                            nc.vector.tensor_copy(out=dst, in_=ps[:, 0:nr])
                        else:
                            drain(dst, ps[:, 0:nr])
                        return
                raise AssertionError((n, r0, nr, oc))

            stores = []  # (n, oc, r0, nr, tile) in data-readiness order
            for n in range(N_IMG):
                if n < N_IMG - 1:
                    parts = [
                        make_parts(n, oc, [(0, 24), (24, 32)]) for oc in range(2)
                    ]
                    if n == 0:
                        # blocks 0-1 of oc0 first (oc1's weights are still
                        # being binarized, and a block that waits on a sign
                        # must never head-block a ready group on the in-order
                        # PE queue), then alternate
                        groups = [(0, 0), (1, 0), (0, 1), (1, 1)]
                        groups += [
                            (b, oc) for b in range(2, NBLK) for oc in range(2)
                        ]
                    else:
                        # oc alternates per block: halves the PE demand rate
                        # on not-yet-signed rows
                        groups = [(b, oc) for b in range(NBLK) for oc in range(2)]
                    for b, oc in groups:
                        conv_group(n, BLK * b, BLK, oc, parts[oc])
                    order = [(0, 0), (1, 0), (0, 1), (1, 1)]
                else:
                    # oc-major: oc1 finishes last, alone, in a descending
                    # ladder of ever-smaller parts whose stores trigger as
                    # each drain lands; the non-overlappable tail is a 4-row
                    # drain plus a 4-row store
                    parts = [
                        make_parts(n, 0, [(0, 24), (24, 32)]),
                        make_parts(n, 1, [(0, 40), (40, 8), (48, 4), (52, 4)]),
                    ]
                    for b in range(NBLK):
                        conv_group(n, BLK * b, BLK, 0, parts[0])
                    for b in range(NBLK - 1):
                        conv_group(n, BLK * b, BLK, 1, parts[1])
                    conv_group(n, 48, 4, 1, parts[1])
                    # the very last drain runs on the (idle) scalar engine,
                    # in parallel with the vector engine's previous drain
                    conv_group(n, 52, 4, 1, parts[1], drain=nc.scalar.copy)
                    order = [(0, 0), (0, 1)] + [(1, i) for i in range(4)]
                for oc, pi in order:
                    r0, nr, tile = parts[oc][pi]
                    stores.append((n, oc, r0, nr, tile))
            # the last three stores issue from three different sequencers so
            # their trigger paths overlap instead of serializing behind one
            # in-order queue
            tail_q = {len(stores) - 2: nc.gpsimd, len(stores) - 1: nc.scalar}
            for i, (n, oc, r0, nr, tile) in enumerate(stores):
                eng = tail_q.get(i, nc.sync)
                eng.dma_start(
                    y_ap[n, oc * 128 : (oc + 1) * 128][:, r0 : r0 + nr], tile
                )
    nc.compile()
    return nc


def _bf16_view(a: np.ndarray) -> np.ndarray:
    """High 2 bytes of each f32 (little-endian) as bfloat16 — a pure byte
    gather; no value arithmetic. sign(bf16_view(v)) == sign(v) for every
    normal f32."""
    import ml_dtypes

    a = np.ascontiguousarray(a, dtype=np.float32)
    hi = a.view(np.uint16).reshape(*a.shape, 2)[..., 1]
    return np.ascontiguousarray(hi).view(ml_dtypes.bfloat16)


def _prep_weights(weights: np.ndarray) -> np.ndarray:
    w = np.asarray(weights, dtype=np.float32).reshape(COUT, CIN, 3, 3)
    # [o, c, dh, dw] -> [o//128, c%128, c//128, tap, o%128]
    w = w.reshape(2, 128, 2, 128, 9)  # [o2, o, c2, c, tap]
    w = w.transpose(0, 3, 2, 4, 1)  # [o2, c, c2, tap, o]
    w = np.ascontiguousarray(w)
    return _bf16_view(w) if W_BF16 else w


def kernel(x: np.ndarray, weights: np.ndarray) -> np.ndarray:
    global LAST_RESULTS
    if "nc" not in _cache:
        _cache["nc"] = _build_nc()
    nc = _cache["nc"]

    x = np.ascontiguousarray(np.asarray(x, dtype=np.float32))
    if X_BF16:
        x = _bf16_view(x)
    wprep = _prep_weights(weights)
    in_maps = [
        {"x": x[i * N_IMG : (i + 1) * N_IMG], "w": wprep} for i in range(N_CORES)
    ]
    res = run_bass_kernel_spmd(
        nc, in_maps, core_ids=list(range(N_CORES)), trace=TRACE
    )
    LAST_RESULTS = res
    return np.concatenate([r["y"] for r in res.results], axis=0).astype(
        np.float32
    )


# revision 32
# speedup vs baseline: 1.0137x; 1.0137x over previous
"""HardBinaryConv Trainium2 kernel.

Computes y = conv2d(sign(x), sign(w)) for x [32,256,56,56] f32, w flat
[256*256*3*3, 1] f32, 3x3 kernel, stride 1, pad 1 (the STE forward pass of
reference.py).

Strategy: data-parallel over batch across 8 cores (4 images/core), weights
replicated. Per core: binarize x on the scalar engine (Sign) to fp8e4
(+-1/0 exact) into zero-padded 58x58 SBUF images, both 128-channel chunks
packed [128, 2, 3376] (16B-aligned stride for DoubleRow); binarize the
host-relaid-out weights to fp8. Conv = 9 accumulating fp8 DoubleRow
matmuls (256-channel contraction per pass, one per 3x3 tap) per PSUM tile
of [128 out-ch, 8 rows x 56 cols]; the rhs streams a strided [2, 8, 56]
window of the padded image, so horizontal taps are plain flat offsets and
padding columns are never computed.

The tensor engine (504 groups x 448 output rows at the fp8 DoubleRow
rate, ~47.0us) is the binding resource; the schedule holds it at 100%
from ~10us on, and everything else is arranged to shorten the lead-in,
the tail, and HBM traffic:
 - y is written as f16 (conv of +-1/0 values is an exact small integer;
   f16 holds integers exactly to 2048) and widened to f32 on the host.
 - x and w are uploaded as the high 2 bytes of each f32 (a pure
   byte-gather view = bf16 truncation, no host arithmetic). sign() of a
   truncated f32 equals sign() of the original for every normal float,
   so the device result is unchanged while input HBM traffic halves.
 - w is split into two per-oc-chunk tensors, each loaded and binarized
   in two tap-slices, so the first matmul group waits on a quarter of
   the weight bytes; the first two x chunks stream before the weights.
 - image 0 arrives in 8-row chunks whose boundaries match the 8-row
   output blocks (each sign() unlocks the next block) and its first two
   blocks run oc0-only while oc1's weights are still being binarized;
   later images use coarser chunks and alternate oc per block.
 - output staging is one SBUF tile per store so a store's dependency is
   exactly the drains that feed it; all stores are issued after every
   load is queued (input never waits on output at the DMA engines), and
   image 3 runs oc-major with a descending ladder of store sizes whose
   last three triggers leave from three different sequencers - the
   non-overlappable tail is a 4-row drain plus a 4-row store.
 - a bridge of tiny self-referential matmuls keeps the PE busy from
   t~0.5 to the first real matmul so the p-state ramp is complete.

Since all matmul operands are exactly +-1/0 (sums of <=2304 of them are
exact integers in f32 PSUM and f16 output), the result is bit-exact vs
the reference (rel err 0.0).
"""

import numpy as np

import concourse.bass as bass
import concourse.bacc as bacc
import concourse.mybir as mybir
from concourse.tile import TileContext
from concourse.bass_utils import run_bass_kernel_spmd

N_CORES = 8
N_IMG = 4          # images per core
CIN = 256
COUT = 256
H = W = 56
WP = 58            # padded width
BASE = 2           # guard elements in front of the padded image
CSTRIDE = 3376     # per-c-chunk stride in the padded tile (16B aligned for fp8)
BLK = 8            # output rows per PSUM tile
NBLK = 7           # 56 / 8
NSPAN = BLK * WP   # 464 <= 512 (one PSUM bank in f32)

# x row chunks; block b needs rows <= 8b+8. Image 0 arrives in 8-row
# pieces (each sign() unlocks the next block while the pipeline fills);
# later images use coarser chunks (fewer instructions, pipeline has slack).
ROWCHUNKS0 = [(0, 9), (9, 8), (17, 8), (25, 8), (33, 8), (41, 8), (49, 7)]
ROWCHUNKS = [(0, 9), (9, 16), (25, 16), (41, 15)]

TRACE = False          # set by test.py to get a profile
LAST_RESULTS = None    # BassKernelResults of the last run (when TRACE)

W_BF16 = True          # upload weights as truncated-f32 (bf16 byte view)
X_BF16 = True          # upload x as truncated-f32 (bf16 byte view)
Y_F16 = True           # store y as f16 (exact for this op), widen on host
N_BRIDGE = 270         # warm-up matmuls bridging t~0.5us .. first real matmul

_cache = {}


def _build_nc():
    nc = bacc.Bacc("TRN2", num_devices=N_CORES)
    f32 = mybir.dt.float32
    bdt = mybir.dt.float8e4
    xdt = mybir.dt.bfloat16 if X_BF16 else f32
    wdt = mybir.dt.bfloat16 if W_BF16 else f32
    ydt = mybir.dt.float16 if Y_F16 else f32

    x_t = nc.dram_tensor("x", [N_IMG, CIN, H, W], xdt, kind="ExternalInput")
    # host-prepped weight layout: [o-chunk, c%128, c//128, tap(3*dh+dw), o]
    w_t = nc.dram_tensor("w", [2, 128, 2, 9, 128], wdt, kind="ExternalInput")
    y_t = nc.dram_tensor("y", [N_IMG, COUT, H, W], ydt, kind="ExternalOutput")
    x_ap, w_ap, y_ap = x_t.ap(), w_t.ap(), y_t.ap()

    chunks = [(0, r0, nr) for r0, nr in ROWCHUNKS0] + [
        (n, r0, nr) for n in range(1, N_IMG) for r0, nr in ROWCHUNKS
    ]

    with TileContext(nc) as tc:
        with (
            tc.tile_pool(name="persist", bufs=1) as persist,
            tc.tile_pool(name="stq", bufs=12) as stq,
            tc.tile_pool(name="outp", bufs=1) as outp,
            tc.tile_pool(name="psum", bufs=7, space="PSUM") as psump,
            tc.tile_pool(name="psbr", bufs=1, space="PSUM") as psbr,
        ):
            # --- PE p-state warm-up bridge: tiny matmuls on a zeroed tile ---
            dz = persist.tile([128, 2, 192], bdt, name="dz")
            nc.gpsimd.memset(dz, 0.0)
            psd = psbr.tile([128, 64], f32, name="psd")
            for _ in range(N_BRIDGE):
                nc.tensor.matmul(
                    psd,
                    dz[:, :, 0:128],
                    dz[:, :, 128:192],
                    start=True,
                    stop=True,
                    perf_mode=mybir.MatmulPerfMode.DoubleRow,
                )

            # --- padded binarized images: [128, cc=2, 3376] ---
            xp = []
            for n in range(N_IMG):
                p = persist.tile([128, 2, CSTRIDE], bdt, name=f"xp_{n}")
                # zero guard/border cells: front guard + top row + row1-col0;
                # row56-col57 + bottom row + back guard; and the interleaved
                # (col57, next-row col0) pairs of interior rows
                nc.gpsimd.memset(p[:, :, 0 : BASE + WP + 1], 0.0)
                nc.gpsimd.memset(p[:, :, BASE + 57 * WP - 1 : CSTRIDE], 0.0)
                pairs = p[:, :, BASE + WP + 57 : BASE + 56 * WP + 57]
                pairs = pairs.rearrange("p k (r c) -> p k r c", c=WP)[:, :, :, 0:2]
                nc.gpsimd.memset(pairs, 0.0)
                xp.append(p)

            def load_chunk(n, r0, nr):
                src = x_ap[n].rearrange("(k p) h w -> p k h w", p=128)
                xf = stq.tile([128, 2, 16, W], xdt, name="xf", tag="xf")
                nc.sync.dma_start(xf[:, :, 0:nr], src[:, :, r0 : r0 + nr])
                return xf

            def sign_chunk(n, r0, nr, xf):
                interior = xp[n][:, :, BASE + WP + 1 : BASE + WP + 1 + H * WP]
                interior = interior.rearrange("p k (r c) -> p k r c", c=WP)[
                    :, :, :, 0:W
                ]
                nc.scalar.sign(interior[:, :, r0 : r0 + nr], xf[:, :, 0:nr])

            # lead-in critical chain: the first x chunk loads first (its sign
            # runs while the weights stream in); each per-oc weight tensor
            # arrives and is signed in two tap-halves so the first matmul of
            # a group starts as soon as its early taps are binarized
            wf = [
                persist.tile([128, 2, 9, 128], wdt, name=f"wf{oc}")
                for oc in range(2)
            ]
            wb = [
                persist.tile([128, 2, 9, 128], bdt, name=f"wb{oc}")
                for oc in range(2)
            ]
            def load_w(oc, taps):
                nc.sync.dma_start(wf[oc][:, :, taps], w_ap[oc][:, :, taps])
                nc.scalar.sign(wb[oc][:, :, taps], wf[oc][:, :, taps])

            xf0 = load_chunk(*chunks[0])
            sign_chunk(*chunks[0], xf0)
            # the second x chunk streams (and signs) before the weights so
            # neither of the first two oc0 blocks ever waits on an x sign
            sign_chunk(*chunks[1], load_chunk(*chunks[1]))
            load_w(0, slice(0, 7))
            load_w(0, slice(7, 9))
            load_w(1, slice(0, 7))
            load_w(1, slice(7, 9))
            for ch in chunks[2:]:
                sign_chunk(*ch, load_chunk(*ch))

            # output staging is split into per-store tiles (one DMA each) so
            # a store's dependency is exactly the drains that feed it, not
            # the whole image plane; the final tile of img3-oc1 is 4 rows so
            # the only non-overlappable tail is a 4-row drain + 4-row store
            def make_parts(n, oc, bounds):
                return [
                    (
                        r0,
                        nr,
                        outp.tile(
                            [128, nr, W], ydt, name=f"ob{n}_{oc}_{r0}"
                        ),
                    )
                    for r0, nr in bounds
                ]

            # --- conv: 9 accumulating tap matmuls per (img, row-range, oc) ---
            # border taps are trimmed: output row 0 / row 55 / col 0 / col 55
            # take only zeros from the dh=0 / dh=2 / dw=0 / dw=2 taps (the
            # guard cells), so those rows/cols are simply not streamed. The
            # always-full (dh=1, dw=1) tap goes first with start=True to
            # initialize the whole PSUM footprint.
            TAPS = [(1, 1)] + [
                (dh, dw) for dh in range(3) for dw in range(3) if (dh, dw) != (1, 1)
            ]

            def conv_group(n, r0, nr, oc, parts, drain=None):
                ps = psump.tile([128, BLK, W], f32, name="ps", tag="ps")
                for i, (dh, dw) in enumerate(TAPS):
                    t = 3 * dh + dw
                    lo = 1 if (r0 == 0 and dh == 0) else 0
                    hi = nr - 1 if (r0 + nr == H and dh == 2) else nr
                    cl, cr = (1, W) if dw == 0 else ((0, W - 1) if dw == 2 else (0, W))
                    s = BASE + (r0 + lo + dh) * WP + dw - 1
                    rhs = xp[n][
                        :, :, s : s + (hi - lo) * WP
                    ].rearrange("p k (r c) -> p k r c", c=WP)[..., cl + 1 : cr + 1]
                    nc.tensor.matmul(
                        ps[:, lo:hi, cl:cr],
                        wb[oc][:, :, t],
                        rhs,
                        start=(i == 0),
                        stop=(i == 8),
                        perf_mode=mybir.MatmulPerfMode.DoubleRow,
                    )
                for p0, pn, tile in parts:
                    if p0 <= r0 and r0 + nr <= p0 + pn:
                        dst = tile[:, r0 - p0 : r0 - p0 + nr, :]
                        if drain is None:
                            nc.vector.tensor_copy(out=dst, in_=ps[:, 0:nr])
                        else:
                            drain(dst, ps[:, 0:nr])
                        return
                raise AssertionError((n, r0, nr, oc))

            stores = []  # (n, oc, r0, nr, tile) in data-readiness order
            for n in range(N_IMG):
                if n < N_IMG - 1:
                    parts = [
                        make_parts(n, oc, [(0, 24), (24, 32)]) for oc in range(2)
                    ]
                    if n == 0:
                        # blocks 0-1 of oc0 first (oc1's weights are still
                        # being binarized, and a block that waits on a sign
                        # must never head-block a ready group on the in-order
                        # PE queue), then alternate
                        groups = [(0, 0), (1, 0), (0, 1), (1, 1)]
                        groups += [
                            (b, oc) for b in range(2, NBLK) for oc in range(2)
                        ]
                    else:
                        # oc alternates per block: halves the PE demand rate
                        # on not-yet-signed rows
                        groups = [(b, oc) for b in range(NBLK) for oc in range(2)]
                    for b, oc in groups:
                        conv_group(n, BLK * b, BLK, oc, parts[oc])
                    order = [(0, 0), (1, 0), (0, 1), (1, 1)]
                else:
                    # oc-major: oc1 finishes last, alone, in a descending
                    # ladder of ever-smaller parts whose stores trigger as
                    # each drain lands; the non-overlappable tail is a 4-row
                    # drain plus a 4-row store
                    parts = [
                        make_parts(n, 0, [(0, 24), (24, 32)]),
                        make_parts(n, 1, [(0, 40), (40, 8), (48, 4), (52, 4)]),
                    ]
                    for b in range(NBLK):
                        conv_group(n, BLK * b, BLK, 0, parts[0])
                    for b in range(NBLK - 1):
                        conv_group(n, BLK * b, BLK, 1, parts[1])
                    conv_group(n, 48, 4, 1, parts[1])
                    # the very last drain runs on the (idle) scalar engine,
                    # in parallel with the vector engine's previous drain
                    conv_group(n, 52, 4, 1, parts[1], drain=nc.scalar.copy)
                    order = [(0, 0), (0, 1)] + [(1, i) for i in range(4)]
                for oc, pi in order:
                    r0, nr, tile = parts[oc][pi]
                    stores.append((n, oc, r0, nr, tile))
            # the last three stores issue from three different sequencers so
            # their trigger paths overlap instead of serializing behind one
            # in-order queue
            tail_q = {len(stores) - 2: nc.gpsimd, len(stores) - 1: nc.scalar}
            for i, (n, oc, r0, nr, tile) in enumerate(stores):
                eng = tail_q.get(i, nc.sync)
                eng.dma_start(
                    y_ap[n, oc * 128 : (oc + 1) * 128][:, r0 : r0 + nr], tile
                )
    nc.compile()
    return nc


def _bf16_view(a: np.ndarray) -> np.ndarray:
    """High 2 bytes of each f32 (little-endian) as bfloat16 — a pure byte
    gather; no value arithmetic. sign(bf16_view(v)) == sign(v) for every
    normal f32."""
    import ml_dtypes

    a = np.ascontiguousarray(a, dtype=np.float32)
    hi = a.view(np.uint16).reshape(*a.shape, 2)[..., 1]
    return np.ascontiguousarray(hi).view(ml_dtypes.bfloat16)


def _prep_weights(weights: np.ndarray) -> np.ndarray:
    w = np.asarray(weights, dtype=np.float32).reshape(COUT, CIN, 3, 3)
    # [o, c, dh, dw] -> [o//128, c%128, c//128, tap, o%128]
    w = w.reshape(2, 128, 2, 128, 9)  # [o2, o, c2, c, tap]
    w = w.transpose(0, 3, 2, 4, 1)  # [o2, c, c2, tap, o]
    w = np.ascontiguousarray(w)
    return _bf16_view(w) if W_BF16 else w


def kernel(x: np.ndarray, weights: np.ndarray) -> np.ndarray:
    global LAST_RESULTS
    if "nc" not in _cache:
        _cache["nc"] = _build_nc()
    nc = _cache["nc"]

    x = np.ascontiguousarray(np.asarray(x, dtype=np.float32))
    if X_BF16:
        x = _bf16_view(x)
    wprep = _prep_weights(weights)
    in_maps = [
        {"x": x[i * N_IMG : (i + 1) * N_IMG], "w": wprep} for i in range(N_CORES)
    ]
    res = run_bass_kernel_spmd(
        nc, in_maps, core_ids=list(range(N_CORES)), trace=TRACE
    )
    LAST_RESULTS = res
    return np.concatenate([r["y"] for r in res.results], axis=0).astype(
        np.float32
    )


# revision 34
# speedup vs baseline: 1.0159x; 1.0022x over previous
"""HardBinaryConv Trainium2 kernel.

Computes y = conv2d(sign(x), sign(w)) for x [32,256,56,56] f32, w flat
[256*256*3*3, 1] f32, 3x3 kernel, stride 1, pad 1 (the STE forward pass of
reference.py).

Strategy: data-parallel over batch across 8 cores (4 images/core), weights
replicated. Per core: binarize x on the scalar engine (Sign) to fp8e4
(+-1/0 exact) into zero-padded 58x58 SBUF images, both 128-channel chunks
packed [128, 2, 3376] (16B-aligned stride for DoubleRow); binarize the
host-relaid-out weights to fp8. Conv = 9 accumulating fp8 DoubleRow
matmuls (256-channel contraction per pass, one per 3x3 tap) per PSUM tile
of [128 out-ch, 8 rows x 56 cols]; the rhs streams a strided [2, 8, 56]
window of the padded image, so horizontal taps are plain flat offsets and
padding columns are never computed.

The tensor engine (504 groups x 448 output rows at the fp8 DoubleRow
rate, ~47.0us) is the binding resource; the schedule holds it at 100%
from ~10us on, and everything else is arranged to shorten the lead-in,
the tail, and HBM traffic:
 - y is written as f16 (conv of +-1/0 values is an exact small integer;
   f16 holds integers exactly to 2048) and widened to f32 on the host.
 - x and w are uploaded as the high 2 bytes of each f32 (a pure
   byte-gather view = bf16 truncation, no host arithmetic). sign() of a
   truncated f32 equals sign() of the original for every normal float,
   so the device result is unchanged while input HBM traffic halves.
 - w is split into two per-oc-chunk tensors, each loaded and binarized
   in two tap-slices, so the first matmul group waits on a quarter of
   the weight bytes; the first two x chunks stream before the weights.
 - image 0 arrives in 8-row chunks whose boundaries match the 8-row
   output blocks (each sign() unlocks the next block) and its first two
   blocks run oc0-only while oc1's weights are still being binarized;
   later images use coarser chunks and alternate oc per block.
 - output staging is one SBUF tile per store so a store's dependency is
   exactly the drains that feed it; all stores are issued after every
   load is queued (input never waits on output at the DMA engines), and
   image 3 runs oc-major with a descending ladder of store sizes whose
   last three triggers leave from three different sequencers - the
   non-overlappable tail is a 4-row drain plus a 4-row store.
 - a bridge of tiny self-referential matmuls keeps the PE busy from
   t~0.5 to the first real matmul so the p-state ramp is complete.

Since all matmul operands are exactly +-1/0 (sums of <=2304 of them are
exact integers in f32 PSUM and f16 output), the result is bit-exact vs
the reference (rel err 0.0).
"""

import numpy as np

import concourse.bass as bass
import concourse.bacc as bacc
import concourse.mybir as mybir
from concourse.tile import TileContext
from concourse.bass_utils import run_bass_kernel_spmd

N_CORES = 8
N_IMG = 4          # images per core
CIN = 256
COUT = 256
H = W = 56
WP = 58            # padded width
BASE = 2           # guard elements in front of the padded image
CSTRIDE = 3376     # per-c-chunk stride in the padded tile (16B aligned for fp8)
BLK = 8            # output rows per PSUM tile
NBLK = 7           # 56 / 8
NSPAN = BLK * WP   # 464 <= 512 (one PSUM bank in f32)

# x row chunks; block b needs rows <= 8b+8. Image 0 arrives in 8-row
# pieces (each sign() unlocks the next block while the pipeline fills);
# later images use coarser chunks (fewer instructions, pipeline has slack).
ROWCHUNKS0 = [(0, 9), (9, 8), (17, 8), (25, 8), (33, 8), (41, 8), (49, 7)]
ROWCHUNKS = [(0, 9), (9, 16), (25, 16), (41, 15)]

TRACE = False          # set by test.py to get a profile
LAST_RESULTS = None    # BassKernelResults of the last run (when TRACE)

W_BF16 = True          # upload weights as truncated-f32 (bf16 byte view)
X_BF16 = True          # upload x as truncated-f32 (bf16 byte view)
Y_F16 = True           # store y as f16 (exact for this op), widen on host
N_BRIDGE = 270         # warm-up matmuls bridging t~0.5us .. first real matmul

_cache = {}


def _build_nc():
    nc = bacc.Bacc("TRN2", num_devices=N_CORES)
    f32 = mybir.dt.float32
    bdt = mybir.dt.float8e4
    xdt = mybir.dt.bfloat16 if X_BF16 else f32
    wdt = mybir.dt.bfloat16 if W_BF16 else f32
    ydt = mybir.dt.float16 if Y_F16 else f32

    x_t = nc.dram_tensor("x", [N_IMG, CIN, H, W], xdt, kind="ExternalInput")
    # host-prepped weight layout: [o-chunk, c%128, c//128, tap(3*dh+dw), o]
    w_t = nc.dram_tensor("w", [2, 128, 2, 9, 128], wdt, kind="ExternalInput")
    y_t = nc.dram_tensor("y", [N_IMG, COUT, H, W], ydt, kind="ExternalOutput")
    x_ap, w_ap, y_ap = x_t.ap(), w_t.ap(), y_t.ap()

    chunks = [(0, r0, nr) for r0, nr in ROWCHUNKS0] + [
        (n, r0, nr) for n in range(1, N_IMG) for r0, nr in ROWCHUNKS
    ]

    with TileContext(nc) as tc:
        with (
            tc.tile_pool(name="persist", bufs=1) as persist,
            tc.tile_pool(name="stq", bufs=12) as stq,
            tc.tile_pool(name="outp", bufs=1) as outp,
            tc.tile_pool(name="psum", bufs=7, space="PSUM") as psump,
            tc.tile_pool(name="psbr", bufs=1, space="PSUM") as psbr,
        ):
            # --- PE p-state warm-up bridge: tiny matmuls on a zeroed tile ---
            dz = persist.tile([128, 2, 192], bdt, name="dz")
            nc.gpsimd.memset(dz, 0.0)
            psd = psbr.tile([128, 64], f32, name="psd")
            for _ in range(N_BRIDGE):
                nc.tensor.matmul(
                    psd,
                    dz[:, :, 0:128],
                    dz[:, :, 128:192],
                    start=True,
                    stop=True,
                    perf_mode=mybir.MatmulPerfMode.DoubleRow,
                )

            # --- padded binarized images: [128, cc=2, 3376] ---
            xp = []
            for n in range(N_IMG):
                p = persist.tile([128, 2, CSTRIDE], bdt, name=f"xp_{n}")
                # zero guard/border cells: front guard + top row + row1-col0;
                # row56-col57 + bottom row + back guard; and the interleaved
                # (col57, next-row col0) pairs of interior rows
                nc.gpsimd.memset(p[:, :, 0 : BASE + WP + 1], 0.0)
                nc.gpsimd.memset(p[:, :, BASE + 57 * WP - 1 : CSTRIDE], 0.0)
                pairs = p[:, :, BASE + WP + 57 : BASE + 56 * WP + 57]
                pairs = pairs.rearrange("p k (r c) -> p k r c", c=WP)[:, :, :, 0:2]
                nc.gpsimd.memset(pairs, 0.0)
                xp.append(p)

            def load_chunk(n, r0, nr):
                src = x_ap[n].rearrange("(k p) h w -> p k h w", p=128)
                xf = stq.tile([128, 2, 16, W], xdt, name="xf", tag="xf")
                nc.sync.dma_start(xf[:, :, 0:nr], src[:, :, r0 : r0 + nr])
                return xf

            def sign_chunk(n, r0, nr, xf):
                interior = xp[n][:, :, BASE + WP + 1 : BASE + WP + 1 + H * WP]
                interior = interior.rearrange("p k (r c) -> p k r c", c=WP)[
                    :, :, :, 0:W
                ]
                nc.scalar.sign(interior[:, :, r0 : r0 + nr], xf[:, :, 0:nr])

            # lead-in critical chain: the first x chunk loads first (its sign
            # runs while the weights stream in); each per-oc weight tensor
            # arrives and is signed in two tap-halves so the first matmul of
            # a group starts as soon as its early taps are binarized
            wf = [
                persist.tile([128, 2, 9, 128], wdt, name=f"wf{oc}")
                for oc in range(2)
            ]
            wb = [
                persist.tile([128, 2, 9, 128], bdt, name=f"wb{oc}")
                for oc in range(2)
            ]
            def load_w(oc, taps):
                nc.sync.dma_start(wf[oc][:, :, taps], w_ap[oc][:, :, taps])
                nc.scalar.sign(wb[oc][:, :, taps], wf[oc][:, :, taps])

            xf0 = load_chunk(*chunks[0])
            sign_chunk(*chunks[0], xf0)
            # the second x chunk streams (and signs) before the weights so
            # neither of the first two oc0 blocks ever waits on an x sign
            sign_chunk(*chunks[1], load_chunk(*chunks[1]))
            load_w(0, slice(0, 7))
            load_w(0, slice(7, 9))
            load_w(1, slice(0, 7))
            load_w(1, slice(7, 9))
            for ch in chunks[2:]:
                sign_chunk(*ch, load_chunk(*ch))

            # output staging is split into per-store tiles (one DMA each) so
            # a store's dependency is exactly the drains that feed it, not
            # the whole image plane; the final tile of img3-oc1 is 4 rows so
            # the only non-overlappable tail is a 4-row drain + 4-row store
            def make_parts(n, oc, bounds):
                return [
                    (
                        r0,
                        nr,
                        outp.tile(
                            [128, nr, W], ydt, name=f"ob{n}_{oc}_{r0}"
                        ),
                    )
                    for r0, nr in bounds
                ]

            # --- conv: 9 accumulating tap matmuls per (img, row-range, oc) ---
            # border taps are trimmed: output row 0 / row 55 / col 0 / col 55
            # take only zeros from the dh=0 / dh=2 / dw=0 / dw=2 taps (the
            # guard cells), so those rows/cols are simply not streamed. The
            # always-full (dh=1, dw=1) tap goes first with start=True to
            # initialize the whole PSUM footprint.
            TAPS = [(1, 1)] + [
                (dh, dw) for dh in range(3) for dw in range(3) if (dh, dw) != (1, 1)
            ]

            def conv_group(n, r0, nr, oc, parts, drain=None):
                ps = psump.tile([128, BLK, W], f32, name="ps", tag="ps")
                for i, (dh, dw) in enumerate(TAPS):
                    t = 3 * dh + dw
                    lo = 1 if (r0 == 0 and dh == 0) else 0
                    hi = nr - 1 if (r0 + nr == H and dh == 2) else nr
                    cl, cr = (1, W) if dw == 0 else ((0, W - 1) if dw == 2 else (0, W))
                    s = BASE + (r0 + lo + dh) * WP + dw - 1
                    rhs = xp[n][
                        :, :, s : s + (hi - lo) * WP
                    ].rearrange("p k (r c) -> p k r c", c=WP)[..., cl + 1 : cr + 1]
                    nc.tensor.matmul(
                        ps[:, lo:hi, cl:cr],
                        wb[oc][:, :, t],
                        rhs,
                        start=(i == 0),
                        stop=(i == 8),
                        perf_mode=mybir.MatmulPerfMode.DoubleRow,
                    )
                for p0, pn, tile in parts:
                    if p0 <= r0 and r0 + nr <= p0 + pn:
                        dst = tile[:, r0 - p0 : r0 - p0 + nr, :]
                        if drain is None:
                            nc.vector.tensor_copy(out=dst, in_=ps[:, 0:nr])
                        else:
                            drain(dst, ps[:, 0:nr])
                        return
                raise AssertionError((n, r0, nr, oc))

            stores = []  # (n, oc, r0, nr, tile) in data-readiness order
            for n in range(N_IMG):
                if n < N_IMG - 1:
                    parts = [
                        make_parts(n, oc, [(0, 24), (24, 32)]) for oc in range(2)
                    ]
                    if n == 0:
                        # blocks 0-1 of oc0 first (oc1's weights are still
                        # being binarized, and a block that waits on a sign
                        # must never head-block a ready group on the in-order
                        # PE queue), then alternate
                        groups = [(0, 0), (1, 0), (0, 1), (1, 1)]
                        groups += [
                            (b, oc) for b in range(2, NBLK) for oc in range(2)
                        ]
                    else:
                        # oc alternates per block: halves the PE demand rate
                        # on not-yet-signed rows
                        groups = [(b, oc) for b in range(NBLK) for oc in range(2)]
                    for b, oc in groups:
                        conv_group(n, BLK * b, BLK, oc, parts[oc])
                    order = [(0, 0), (1, 0), (0, 1), (1, 1)]
                else:
                    # oc-major: oc1 finishes last, alone, in a descending
                    # ladder of ever-smaller parts whose stores trigger as
                    # each drain lands; the non-overlappable tail is a 4-row
                    # drain plus a 4-row store
                    parts = [
                        make_parts(n, 0, [(0, 24), (24, 32)]),
                        make_parts(
                            n,
                            1,
                            [(0, 16), (16, 16), (32, 16), (48, 4), (52, 4)],
                        ),
                    ]
                    for b in range(NBLK):
                        conv_group(n, BLK * b, BLK, 0, parts[0])
                    for b in range(NBLK - 1):
                        conv_group(n, BLK * b, BLK, 1, parts[1])
                    conv_group(n, 48, 4, 1, parts[1])
                    # the very last drain runs on the (idle) scalar engine,
                    # in parallel with the vector engine's previous drain
                    conv_group(n, 52, 4, 1, parts[1], drain=nc.scalar.copy)
                    order = [(0, 0), (0, 1)] + [(1, i) for i in range(5)]
                for oc, pi in order:
                    r0, nr, tile = parts[oc][pi]
                    stores.append((n, oc, r0, nr, tile))
            # the last three stores issue from three different sequencers so
            # their trigger paths overlap instead of serializing behind one
            # in-order queue
            tail_q = {len(stores) - 2: nc.gpsimd, len(stores) - 1: nc.scalar}
            for i, (n, oc, r0, nr, tile) in enumerate(stores):
                eng = tail_q.get(i, nc.sync)
                eng.dma_start(
                    y_ap[n, oc * 128 : (oc + 1) * 128][:, r0 : r0 + nr], tile
                )
    nc.compile()
    return nc


def _bf16_view(a: np.ndarray) -> np.ndarray:
    """High 2 bytes of each f32 (little-endian) as bfloat16 — a pure byte
    gather; no value arithmetic. sign(bf16_view(v)) == sign(v) for every
    normal f32."""
    import ml_dtypes

    a = np.ascontiguousarray(a, dtype=np.float32)
    hi = a.view(np.uint16).reshape(*a.shape, 2)[..., 1]
    return np.ascontiguousarray(hi).view(ml_dtypes.bfloat16)


def _prep_weights(weights: np.ndarray) -> np.ndarray:
    w = np.asarray(weights, dtype=np.float32).reshape(COUT, CIN, 3, 3)
    # [o, c, dh, dw] -> [o//128, c%128, c//128, tap, o%128]
    w = w.reshape(2, 128, 2, 128, 9)  # [o2, o, c2, c, tap]
    w = w.transpose(0, 3, 2, 4, 1)  # [o2, c, c2, tap, o]
    w = np.ascontiguousarray(w)
    return _bf16_view(w) if W_BF16 else w


def kernel(x: np.ndarray, weights: np.ndarray) -> np.ndarray:
    global LAST_RESULTS
    if "nc" not in _cache:
        _cache["nc"] = _build_nc()
    nc = _cache["nc"]

    x = np.ascontiguousarray(np.asarray(x, dtype=np.float32))
    if X_BF16:
        x = _bf16_view(x)
    wprep = _prep_weights(weights)
    in_maps = [
        {"x": x[i * N_IMG : (i + 1) * N_IMG], "w": wprep} for i in range(N_CORES)
    ]
    res = run_bass_kernel_spmd(
        nc, in_maps, core_ids=list(range(N_CORES)), trace=TRACE
    )
    LAST_RESULTS = res
    return np.concatenate([r["y"] for r in res.results], axis=0).astype(
        np.float32
    )
